# revision 58
# baseline (speedup 1.0000x reference)
"""TAGConv-style GNN encoder (degree-normalized edge aggregation + linear +
L2 row-normalize) on 8 Trainium2 NeuronCores.

Strategy (dst-sharded, fully data-parallel — no collectives):
  - Nodes are sharded by destination: core c owns dst rows [c*NPC, (c+1)*NPC).
  - Host-side graph partitioning (integer index metadata only): lay each
    core's edges out into 128-edge tiles grouped by (aligned WIN-wide dst
    window, src-chunk). The tile schedule is made identical across cores
    (padded to the per-(window,chunk) max) so one SPMD program serves all 8.
  - Norm factorization: every edge instance has weight 1 and
    scale_e = rsqrt(deg_src) * rsqrt(deg_dst).  rsqrt(deg_src) is baked into
    a device-side pre-scaled copy of the node table (hs, internal DRAM);
    rsqrt(deg_dst) is a per-dst-column scale applied at PSUM evacuation.
    The per-edge one-hot is then a pure is_equal {0,1} (pad slots get
    offs=-1 so they never match).
  - Device per core: gpsimd dma_gather (4 SWDGE queues, int16 idxs => table
    split into 4 chunks of 25000 rows) pulls hs rows (bf16) into SBUF tiles
    [128 edges, 128 feat]; DVE builds the one-hot [128 edges, WIN dst-slots];
    TensorE matmul G.T @ onehot accumulates segment sums into PSUM.  Then
    out^T = W1.T @ h^T + W2.T @ agg^T, + bias, L2 row-normalize via a
    ones-matmul partition reduction.  h^T comes from a HW DMA transpose.
    Output is written transposed [128, NPC_padded]; the host transposes/
    concatenates shards.
"""
import os
import numpy as np
import ml_dtypes

import concourse.bass as bass
import concourse.tile as tile
from concourse import mybir, bacc
from concourse.bass_utils import run_bass_kernel_spmd

F32 = mybir.dt.float32
BF16 = mybir.dt.bfloat16
I32 = mybir.dt.int32
I16 = mybir.dt.int16


def _patched_drain_and_barrier(self, tick_clock, wait_clock):
    """Tile's kernel-tail Drain carries one sync-wait per outstanding
    semaphore; the walrus build in this container can't encode more than one
    wait on one instruction. Emit each wait as its own wait_ge instead."""
    nc = self.nc
    probe = nc.sync.nop(nofuse=True)
    wait_clock.add_sem_waits(probe.ins, tile.ScopedClock({None: tick_clock.global_clock}))
    si = probe.ins.sync_info
    waits = list(si.on_wait) if si is not None else []
    if len(waits) > 1:
        si.on_wait.clear()
        sem_by_num = {h.num: h for h in self.sems.allocated().values()}
        for w in waits:
            nc.sync.wait_ge(sem_by_num[w.id], w.wait_value)
    nc.sync.drain()
    nc.all_engine_barrier()
    popped = nc._tile_sem_poison_stack.pop()
    assert popped is self._sem_poison
    nc.clear_and_free_semaphores(list(self.sems.allocated().values()))
    nc.all_engine_barrier()


tile.TileContext._drain_and_barrier = _patched_drain_and_barrier

# this walrus build encodes at most this many sync waits on one instruction
MAX_WAITS = 1


def _split_excess_waits(nc, max_waits=MAX_WAITS):
    """Hoist sync waits beyond the per-instruction ISA budget onto NoOps
    inserted just before the instruction (same engine queue, so ordering
    semantics are identical). Must run AFTER Bacc.compile (its nop-fusion
    passes would re-merge the waits)."""
    for f in nc.m.functions:
        for b in f.blocks:
            ins_list = b.instructions
            out_list = []
            changed = False
            for ins in ins_list:
                si = ins.sync_info
                waits = list(si.on_wait) if si is not None else []
                if len(waits) > max_waits:
                    excess, keep = waits[:-max_waits], waits[-max_waits:]
                    for j in range(0, len(excess), max_waits):
                        nop = mybir.InstNoOp(
                            name=nc.get_next_instruction_name(), ins=[], outs=[])
                        nop.engine = ins.engine
                        nop.sync_info = mybir.SyncInfo(
                            on_wait=excess[j:j + max_waits], on_update=[])
                        out_list.append(nop)
                    ins.sync_info = mybir.SyncInfo(
                        on_wait=keep, on_update=list(si.on_update))
                    changed = True
                out_list.append(ins)
            if changed:
                b.instructions = out_list


# Problem constants (hardcoded: harness contract)
N_NODES = 100000
D = 128
HID = 128
CORES = 8

# Kernel tuning
WIN = 256         # dst window width = segment-matmul N
TILE = 128        # edge slots per tile (= matmul K)
BANK = 512        # PSUM bank width in f32 cols
CHUNK_WINS = 6    # windows per PSUM chunk (6*256 = 1536 cols = 3 banks)
GX = 6            # gather tiles per dma_gather instruction
MERGE = False     # merge gather groups across (window) regions within (chunk,k)
SCH = 4           # src chunks (int16 gather indices => table <= 32767 rows)
QN = 4            # SWDGE queues (desc-gen parallelism across Q7 cores)
PB = 8            # pre-scale pass: x-slices (128 rows each) per block
DDS = 16384       # SWDGE descriptor ring scratch bytes/partition

# crash-bisection toggles (numerics may be wrong with these set)
DBG_NOPREPASS = bool(int(os.environ.get("K_NOPREPASS", "0")))
DBG_NOXPOSE = bool(int(os.environ.get("K_NOXPOSE", "0")))
DBG_NOPB = bool(int(os.environ.get("K_NOPB", "0")))
DBG_NORAF = bool(int(os.environ.get("K_NORAF", "0")))
DBG_NOSEM = bool(int(os.environ.get("K_NOSEM", "0")))


def _prepass_blocks(r_start, r_end):
    """(row0, x_slices, partitions) blocks covering table rows
    [r_start, r_end) for the device-side pre-scale pass. Full blocks are
    PB*128 rows; the tail is a short block plus a partial-partition block."""
    blocks = []
    r = r_start
    while r + PB * 128 <= r_end:
        blocks.append((r, PB, 128))
        r += PB * 128
    rem = r_end - r
    full = rem // 128
    if full:
        blocks.append((r, full, 128))
        r += full * 128
    tail = r_end - r
    if tail:
        blocks.append((r, 1, tail))
    return blocks


def _preprocess(src, dst, n_nodes, npc, cores):
    """Host-side graph partitioning (integer index metadata only)."""
    assert n_nodes % SCH == 0
    cn = n_nodes // SCH
    assert cn < 32768, "src-chunk must fit int16 gather indices"
    src = np.asarray(src).astype(np.int64)
    dst = np.asarray(dst).astype(np.int64)
    deg = np.bincount(dst, minlength=n_nodes)
    degc = np.maximum(deg, 1)

    core_of = dst // npc
    ldst = dst - core_of * npc
    win = ldst // WIN
    kch = src // cn
    n_wins = (npc + WIN - 1) // WIN
    n_codes = n_wins * SCH
    code = win * SCH + kch

    # edges per (core, window, chunk); uniform tiles-per-(w,k) schedule
    cnt = np.zeros((cores, n_codes), np.int64)
    for c in range(cores):
        m = core_of == c
        cnt[c] = np.bincount(code[m], minlength=n_codes)
    tiles_wk = (-(-cnt.max(axis=0) // TILE)).reshape(n_wins, SCH)
    empty = tiles_wk.sum(axis=1) == 0
    tiles_wk[empty, 0] = 1  # every window writes its PSUM cols at least once

    # program tile order: psum-chunk major, then src-chunk, then window
    order = []  # (w, k) per tile
    for p0 in range(0, n_wins, CHUNK_WINS):
        p1 = min(n_wins, p0 + CHUNK_WINS)
        for k in range(SCH):
            for w in range(p0, p1):
                order.extend([(w, k)] * int(tiles_wk[w, k]))
    n_tiles = len(order)
    wk = np.array(order, np.int64)
    win_of_tile = wk[:, 0]
    # first slot of each (w,k) region (regions are contiguous in tile order)
    slot_base = np.full(n_codes, -1, np.int64)
    region_code = np.zeros(n_tiles, np.int64)
    region_start = np.zeros(n_tiles, np.int64)
    for t, (w, k) in enumerate(order):
        c_ = w * SCH + k
        if slot_base[c_] < 0:
            slot_base[c_] = t * TILE
        region_code[t] = c_
        region_start[t] = slot_base[c_] // TILE

    n_slots = n_tiles * TILE

    # gather instruction groups: balanced runs of consecutive tiles sharing
    # a src-chunk. MERGE=False: runs stay within one (w,k) region.
    # MERGE=True: runs span windows within one (psum-chunk, k) — pad slots
    # (gidx=0, offs=-1) contribute nothing, so interior pads are legal.
    def _split_run(k, ta, tb):
        nt = tb - ta
        ng = -(-nt // GX)
        base, rem = divmod(nt, ng)
        a = ta
        for i in range(ng):
            sz = base + (1 if i < rem else 0)
            groups.append((k, a, a + sz))
            a += sz
        assert a == tb

    groups = []  # (k, t_start, t_end)
    t = 0
    for p0 in range(0, n_wins, CHUNK_WINS):
        p1 = min(n_wins, p0 + CHUNK_WINS)
        for k in range(SCH):
            run0 = t
            for w in range(p0, p1):
                nt_r = int(tiles_wk[w, k])
                if not MERGE and nt_r:
                    _split_run(k, t, t + nt_r)
                t += nt_r
            if MERGE and t > run0:
                _split_run(k, run0, t)
    assert t == n_tiles

    padn = n_wins * WIN

    per_core = []
    for c in range(cores):
        m = np.nonzero(core_of == c)[0]
        nm = len(m)
        src_c, win_c, kch_c, code_c, ldst_c = src[m], win[m], kch[m], code[m], ldst[m]
        # group by (w,k), ascending src within the group (gather locality)
        o = np.lexsort((src_c, kch_c, win_c))
        codes_s = code_c[o]
        gstart = np.searchsorted(codes_s, np.arange(n_codes))
        rank = np.arange(nm) - gstart[codes_s]
        slot = slot_base[codes_s] + rank

        gidx = np.zeros(n_slots, np.int16)    # pads: row 0 of the chunk
        offs = np.full(n_slots, -1.0, np.float32)  # pads never match iota

        gidx[slot] = (src_c[o] - kch_c[o] * cn).astype(np.int16)
        offs[slot] = (ldst_c[o] - win_c[o] * WIN).astype(np.float32)

        # per-group real descriptor counts: the gather ucode only processes
        # num_idxs_reg descriptors, so tail pads (gidx 0 / offs -1) are
        # skipped. Groups lie within one (w,k) region (MERGE=False), whose
        # real edges fill slots from the region start.
        real_of_code = cnt[c]
        counts = np.zeros(len(groups), np.int32)
        for gi_, (k_, ta_, tb_) in enumerate(groups):
            code_ = region_code[ta_]
            r0_ = region_start[ta_]
            real = int(real_of_code[code_])
            c_real = int(np.clip(real - (ta_ - r0_) * TILE,
                                 16, (tb_ - ta_) * TILE))
            counts[gi_] = -(-c_real // 16) * 16  # idx wrap-row granularity

        # [n_slots] -> [128, n_tiles]: slot j of tile t at [j, t]
        offs_t = np.ascontiguousarray(
            offs.reshape(n_tiles, TILE).T).astype(ml_dtypes.bfloat16)

        # int16 idx wrap for dma_gather: within-instruction idx i at
        # [i % 16, i // 16], replicated across the 8 16-partition groups.
        # Instruction = run of whole tiles, so per-tile 8-col blocks suffice.
        a = gidx.reshape(n_tiles, 8, 16)          # [t, i//16, i%16]
        wrapped = np.transpose(a, (2, 0, 1)).reshape(16, n_tiles * 8)
        gidx16 = np.ascontiguousarray(np.tile(wrapped, (8, 1)))  # [128, 8*ET]

        # dst-degree clip, permuted so [p, x] = degc[own_row x*128+p]
        nx_d = padn // 128
        degd = np.ones(padn, np.float32)
        degd[:npc] = degc[c * npc:(c + 1) * npc].astype(np.float32)
        degdp = np.ascontiguousarray(degd.reshape(nx_d, 128).T)  # [128, nx_d]

        per_core.append(dict(gidx16=gidx16, offs=offs_t, degdp=degdp,
                             counts=counts.reshape(1, -1)))

    # src-degree clip for the table pre-scale pass, laid out in pre-pass
    # block emission order: column j = j-th x-slice emitted, value
    # degc[r0 + x*pcnt + p] (pad 1.0 on partial-partition tails)
    flat = degc.astype(np.float32)
    cols = []
    for k in range(SCH):
        for (r0, xs, pcnt) in _prepass_blocks(k * cn, (k + 1) * cn):
            for x in range(xs):
                col = np.ones(128, np.float32)
                col[:pcnt] = flat[r0 + x * pcnt: r0 + (x + 1) * pcnt]
                cols.append(col)
    degsp = np.ascontiguousarray(np.stack(cols, axis=1))

    return dict(
        groups=groups,
        n_wins=n_wins,
        n_tiles=n_tiles,
        win_of_tile=win_of_tile,
        per_core=per_core,
        degsp=degsp,
    )


def _build_program(sched, n_nodes, npc, split_waits=True):
    """Build the single SPMD Bass/Tile program (identical for all cores)."""
    n_wins = sched["n_wins"]
    n_tiles = sched["n_tiles"]
    win_of_tile = sched["win_of_tile"]
    cn = n_nodes // SCH
    padn = n_wins * WIN            # padded local dst count (cols of out^T)
    n_chunks = -(-n_wins // CHUNK_WINS)
    nxb = sched["degsp"].shape[1]
    nx_d = padn // 128

    nc = bacc.Bacc("TRN2", target_bir_lowering=False, num_swdge_queues=QN,
                   dynamic_dma_scratch_size=DDS)
    hb = nc.declare_dram_parameter("hb", [n_nodes, D], BF16, isOutput=False)
    hself = nc.declare_dram_parameter("hself", [padn, D], BF16, isOutput=False)
    gidx_p = nc.declare_dram_parameter("gidx16", [TILE, 8 * n_tiles], I16, isOutput=False)
    offs_p = nc.declare_dram_parameter("offs", [TILE, n_tiles], BF16, isOutput=False)
    wt_p = nc.declare_dram_parameter("wt", [2 * D, HID], BF16, isOutput=False)
    bias_p = nc.declare_dram_parameter("bias_c", [HID, 1], F32, isOutput=False)
    degsp_p = nc.declare_dram_parameter("degsp", [128, nxb], F32, isOutput=False)
    degdp_p = nc.declare_dram_parameter("degdp", [128, nx_d], F32, isOutput=False)
    out_p = nc.declare_dram_parameter("out", [HID, padn], F32, isOutput=True)

    # pre-scaled node table (hs = h * rsqrt(deg_src)), device-written
    hs = nc.dram_tensor("hs", [n_nodes, D], BF16, kind="Internal")
    # dst-norm row staging (layout shuffle [128, nx_d] -> flat [padn])
    rd_dram = nc.dram_tensor("rd_dram", [nx_d, 128], BF16, kind="Internal")

    with tile.TileContext(nc) as tc:
        with (
            tc.tile_pool(name="const", bufs=1) as const,
            tc.tile_pool(name="pp", bufs=3) as pp,
            tc.tile_pool(name="g", bufs=12) as gpool,
            tc.tile_pool(name="oh", bufs=10) as ohpool,
            tc.tile_pool(name="nd", bufs=2) as ndpool,
            tc.tile_pool(name="slab", bufs=2) as slab,
            tc.tile_pool(name="y", bufs=6) as ypool,
            tc.tile_pool(name="aggps", bufs=2, space="PSUM") as agg_ps,
            tc.tile_pool(name="scrps", bufs=2, space="PSUM") as scr_ps,
        ):
            # ---- constants / metadata ----
            gidx_sb = const.tile([TILE, 8 * n_tiles], I16)
            nc.sync.dma_start(gidx_sb[:], gidx_p[:])
            offs_sb = const.tile([TILE, n_tiles], BF16)
            nc.sync.dma_start(offs_sb[:], offs_p[:])

            w1_sb = const.tile([D, HID], BF16)
            nc.sync.dma_start(w1_sb[:], wt_p[0:D, :])
            w2_sb = const.tile([D, HID], BF16)
            nc.sync.dma_start(w2_sb[:], wt_p[D:2 * D, :])
            bias_sb = const.tile([HID, 1], F32)
            nc.sync.dma_start(bias_sb[:], bias_p[:])
            ones_sb = const.tile([128, 128], F32)
            nc.vector.memset(ones_sb[:], 1.0)

            # iota replicated per gather-tile: [128, GX, WIN], value = col
            iota_i = const.tile([128, GX, WIN], I32)
            nc.gpsimd.iota(iota_i[:], pattern=[[0, GX], [1, WIN]], base=0,
                           channel_multiplier=0)
            iota_b = const.tile([128, GX, WIN], BF16)
            nc.vector.tensor_copy(iota_b[:], iota_i[:])

            # ---- src-norm: rs[p, x] = rsqrt(degc[x*128+p]) ----
            degs_sb = const.tile([128, nxb], F32)
            nc.sync.dma_start(degs_sb[:], degsp_p[:])
            rs_sb = const.tile([128, nxb], F32)
            if DBG_NORAF:
                nc.vector.reciprocal(rs_sb[:], degs_sb[:])
            else:
                nc.vector.reciprocal_approx_fast(rs_sb[:], degs_sb[:])
            nc.scalar.sqrt(rs_sb[:], rs_sb[:])

            # ---- dst-norm row vector normd1[0, j] = rsqrt(degc_dst[j]) ----
            degd_sb = const.tile([128, nx_d], F32)
            nc.sync.dma_start(degd_sb[:], degdp_p[:])
            rdf_sb = const.tile([128, nx_d], F32)
            if DBG_NORAF:
                nc.vector.reciprocal(rdf_sb[:], degd_sb[:])
            else:
                nc.vector.reciprocal_approx_fast(rdf_sb[:], degd_sb[:])
            nc.scalar.sqrt(rdf_sb[:], rdf_sb[:])
            rd_sb = const.tile([128, nx_d], BF16)
            nc.vector.tensor_copy(rd_sb[:], rdf_sb[:])
            normd1 = const.tile([1, padn], BF16)
            if not DBG_NOPB:
                # normd1[0, x*128+p] = rd_sb[p, x], via a DRAM staging bounce
                # (the write->read hazard is covered by the drain below).
                # Pre-pass DMAs ride the Activation HWDGE queue so they don't
                # serialize behind the big metadata loads on the SP queue.
                nc.scalar.dma_start(
                    rd_dram[:, :].rearrange("x p -> p x"), rd_sb[:])

            # ---- pre-scale pass: hs[r] = hb[r] * rs[r], per src-chunk ----
            # DRAM write->read hazards (hs, rd_dram) are invisible to Tile:
            # after each chunk's stores, drain the SP HWDGE queue (blocks SP
            # until all its issued DMAs complete), then a token DMA whose
            # SBUF tile gates the Pool engine via a tracked dependency right
            # before the chunk's first gather.
            gather_src = hb if DBG_NOPREPASS else hs
            toks = {}
            x0 = 0  # running degsp column (block emission order)
            for k in range(SCH):
                if DBG_NOPREPASS:
                    break
                for (r0, xs, pcnt) in _prepass_blocks(k * cn, (k + 1) * cn):
                    nrows = xs * pcnt
                    tin = pp.tile([128, PB, D], BF16, tag="ppin")
                    nc.scalar.dma_start(
                        tin[:pcnt, :xs, :],
                        hb[r0:r0 + nrows, :].rearrange("(x p) f -> p x f", p=pcnt),
                    )
                    tout = pp.tile([128, PB, D], BF16, tag="ppout")
                    nc.vector.tensor_tensor(
                        out=tout[:pcnt, :xs, :], in0=tin[:pcnt, :xs, :],
                        in1=rs_sb[:pcnt, x0:x0 + xs].unsqueeze(2).broadcast_to(
                            [pcnt, xs, D]),
                        op=mybir.AluOpType.mult)
                    nc.scalar.dma_start(
                        hs[r0:r0 + nrows, :].rearrange("(x p) f -> p x f", p=pcnt),
                        tout[:pcnt, :xs, :],
                    )
                    x0 += xs
                nc.scalar.drain()
                if k == 0 and not DBG_NOPB:
                    # rd_dram bounce was issued before chunk 0's stores
                    nc.scalar.dma_start(
                        normd1[:].rearrange("o (x p) -> o x p", p=128),
                        rd_dram[:, :])
                tok = const.tile([1, 8], BF16, tag=f"tok{k}")
                nc.scalar.dma_start(tok[:], hb[0:1, 0:8])
                toks[k] = tok

            def gate_chunk(k):
                if k in toks:
                    tok2 = const.tile([1, 8], BF16, tag=f"tokc{k}")
                    nc.gpsimd.tensor_copy(tok2[:], toks.pop(k)[:])

            # shared num_idxs registers for dma_gather (one per distinct size)
            ni_regs = {}

            def ni_reg(n):
                if n not in ni_regs:
                    r = nc.gpsimd.alloc_register(f"nireg{len(ni_regs)}")
                    nc.gpsimd.reg_mov(r, n)
                    ni_regs[n] = r
                return ni_regs[n]

            group_by_start = {g[1]: (gi, g) for gi, g in enumerate(sched["groups"])}

            tile_of_chunk = [[] for _ in range(n_chunks)]
            for t in range(n_tiles):
                tile_of_chunk[int(win_of_tile[t]) // CHUNK_WINS].append(t)

            # ---- main loop over dst chunks ----
            for ch in range(n_chunks):
                w0 = ch * CHUNK_WINS
                w1 = min(n_wins, w0 + CHUNK_WINS)
                cw = (w1 - w0) * WIN
                col0 = w0 * WIN
                tlist = tile_of_chunk[ch]
                assert tlist == list(range(tlist[0], tlist[-1] + 1))
                t0c, t1c = tlist[0], tlist[-1] + 1

                # first/last program-order touch per psum bank in this chunk
                bank_of = [(int(win_of_tile[t]) - w0) * WIN // BANK for t in tlist]
                first_of_bank, last_of_bank = {}, {}
                for t, bk in zip(tlist, bank_of):
                    first_of_bank.setdefault(bk, t)
                    last_of_bank[bk] = t

                pagg = agg_ps.tile([128, CHUNK_WINS * WIN], F32, tag="pagg")

                # gather groups: region-aligned runs, up to GX tiles
                g0 = t0c
                while g0 < t1c:
                    gi, (k, ta, gend) = group_by_start[g0]
                    assert ta == g0
                    gt = gend - g0
                    gate_chunk(k)
                    G = gpool.tile([128, GX, D], BF16, tag="G")
                    nc.gpsimd.dma_gather(
                        out_ap=G[:, :gt, :],
                        in_ap=gather_src[k * cn:(k + 1) * cn, :],
                        idxs_ap=gidx_sb[:, 8 * g0:8 * gend],
                        num_idxs=TILE * gt,
                        num_idxs_reg=ni_reg(TILE * gt),
                        elem_size=D,
                        queue_num=gi % QN,
                    )
                    oh = ohpool.tile([128, GX, WIN], BF16, tag="oh")
                    off_bc = offs_sb[:, g0:gend].unsqueeze(2).broadcast_to([128, gt, WIN])
                    nc.vector.tensor_tensor(out=oh[:, :gt, :], in0=off_bc,
                                            in1=iota_b[:, :gt, :],
                                            op=mybir.AluOpType.is_equal)
                    for x in range(gt):
                        t = g0 + x
                        col = (int(win_of_tile[t]) - w0) * WIN
                        bk = bank_of[t - t0c]
                        nc.tensor.matmul(
                            pagg[:, col:col + WIN],
                            lhsT=G[:, x, :],
                            rhs=oh[:, x, :],
                            start=(first_of_bank[bk] == t),
                            stop=(last_of_bank[bk] == t),
                            skip_group_check=True,
                        )
                    g0 = gend

                # dst-norm column scale for this chunk, broadcast to 128 parts
                ndbc = ndpool.tile([128, CHUNK_WINS * WIN], BF16, tag="ndbc")
                if DBG_NOPB:
                    nc.vector.memset(ndbc[:, :cw], 1.0)
                else:
                    nc.gpsimd.partition_broadcast(
                        ndbc[:, :cw], normd1[:, col0:col0 + cw])

                # evacuate agg chunk: cast to bf16 with dst-norm applied
                aggT = slab.tile([128, CHUNK_WINS * WIN], BF16, tag="aggT")
                nc.vector.tensor_tensor(out=aggT[:, :cw], in0=pagg[:, :cw],
                                        in1=ndbc[:, :cw], op=mybir.AluOpType.mult)

                # h^T slab for this chunk's dst rows via HW DMA transpose
                hT = slab.tile([128, CHUNK_WINS * WIN], BF16, tag="hT")
                if DBG_NOXPOSE:
                    nc.vector.memset(hT[:, :cw], 0.0)
                else:
                    nc.sync.dma_start(hT[:, :cw], hself[col0:col0 + cw, :],
                                      transpose=True)

                # out^T = W1.T @ h^T + W2.T @ agg^T ; + bias; L2 normalize; store
                for bs in range(0, cw, BANK):
                    bw = min(BANK, cw - bs)
                    po = scr_ps.tile([128, BANK], F32, tag="scr")
                    nc.tensor.matmul(po[:, :bw], lhsT=w1_sb[:], rhs=hT[:, bs:bs + bw],
                                     start=True, stop=False)
                    nc.tensor.matmul(po[:, :bw], lhsT=w2_sb[:], rhs=aggT[:, bs:bs + bw],
                                     start=False, stop=True)
                    y = ypool.tile([128, BANK], F32, tag="y")
                    nc.scalar.activation(y[:, :bw], po[:, :bw],
                                         mybir.ActivationFunctionType.Identity,
                                         bias=bias_sb[:])
                    z = ypool.tile([128, BANK], F32, tag="z")
                    nc.scalar.square(z[:, :bw], y[:, :bw])
                    pr = scr_ps.tile([128, BANK], F32, tag="scr")
                    nc.tensor.matmul(pr[:, :bw], lhsT=ones_sb[:], rhs=z[:, :bw],
                                     start=True, stop=True)
                    rs = ypool.tile([128, BANK], F32, tag="rs")
                    nc.vector.reciprocal_approx_fast(rs[:, :bw], pr[:, :bw])
                    nc.scalar.sqrt(rs[:, :bw], rs[:, :bw])
                    of = ypool.tile([128, BANK], F32, tag="of")
                    nc.vector.tensor_tensor(out=of[:, :bw], in0=y[:, :bw],
                                            in1=rs[:, :bw], op=mybir.AluOpType.mult)
                    nc.sync.dma_start(out_p[:, col0 + bs:col0 + bs + bw], of[:, :bw])

    nc.finalize()
    if split_waits:
        _split_excess_waits(nc)
    return nc


def _run(h, weight, bias, src, dst, n_nodes, npc, cores, trace=False):
    sched = _preprocess(src, dst, n_nodes, npc, cores)
    nc = _build_program(sched, n_nodes, npc)

    padn = sched["n_wins"] * WIN
    h = np.asarray(h, dtype=np.float32)
    hb = h.astype(ml_dtypes.bfloat16)
    wt = np.asarray(weight, dtype=np.float32).astype(ml_dtypes.bfloat16)
    bias_c = np.ascontiguousarray(np.asarray(bias, dtype=np.float32).reshape(HID, 1))

    in_maps = []
    for c in range(cores):
        pc = sched["per_core"][c]
        hself = np.zeros((padn, D), dtype=ml_dtypes.bfloat16)
        hself[:npc] = hb[c * npc:(c + 1) * npc]
        in_maps.append(dict(
            hb=hb, hself=hself,
            gidx16=pc["gidx16"], offs=pc["offs"],
            wt=wt, bias_c=bias_c,
            degsp=sched["degsp"], degdp=pc["degdp"],
        ))

    res = run_bass_kernel_spmd(nc, in_maps, core_ids=list(range(cores)), trace=trace)
    out = np.empty((cores * npc, HID), dtype=np.float32)
    for c in range(cores):
        out[c * npc:(c + 1) * npc] = res.results[c]["out"][:, :npc].T
    return out, res


def kernel(h, weight, bias, src, dst):
    out, _ = _run(h, weight, bias, src, dst, N_NODES, N_NODES // CORES, CORES)
    return out


# revision 59
# speedup vs baseline: 1.0365x; 1.0365x over previous
"""TAGConv-style GNN encoder (degree-normalized edge aggregation + linear +
L2 row-normalize) on 8 Trainium2 NeuronCores.

Strategy (dst-sharded, fully data-parallel — no collectives):
  - Nodes are sharded by destination: core c owns dst rows [c*NPC, (c+1)*NPC).
  - Host-side graph partitioning (integer index metadata only): lay each
    core's edges out into 128-edge tiles grouped by (aligned WIN-wide dst
    window, src-chunk). The tile schedule is made identical across cores
    (padded to the per-(window,chunk) max) so one SPMD program serves all 8.
  - Norm factorization: every edge instance has weight 1 and
    scale_e = rsqrt(deg_src) * rsqrt(deg_dst).  rsqrt(deg_src) is baked into
    a device-side pre-scaled copy of the node table (hs, internal DRAM);
    rsqrt(deg_dst) is a per-dst-column scale applied at PSUM evacuation.
    The per-edge one-hot is then a pure is_equal {0,1} (pad slots get
    offs=-1 so they never match).
  - Device per core: gpsimd dma_gather (4 SWDGE queues, int16 idxs => table
    split into 4 chunks of 25000 rows) pulls hs rows (bf16) into SBUF tiles
    [128 edges, 128 feat]; DVE builds the one-hot [128 edges, WIN dst-slots];
    TensorE matmul G.T @ onehot accumulates segment sums into PSUM.  Then
    out^T = W1.T @ h^T + W2.T @ agg^T, + bias, L2 row-normalize via a
    ones-matmul partition reduction.  h^T comes from a HW DMA transpose.
    Output is written transposed [128, NPC_padded]; the host transposes/
    concatenates shards.
"""
import os
import numpy as np
import ml_dtypes

import concourse.bass as bass
import concourse.tile as tile
from concourse import mybir, bacc
from concourse.bass_utils import run_bass_kernel_spmd

F32 = mybir.dt.float32
BF16 = mybir.dt.bfloat16
I32 = mybir.dt.int32
I16 = mybir.dt.int16


def _patched_drain_and_barrier(self, tick_clock, wait_clock):
    """Tile's kernel-tail Drain carries one sync-wait per outstanding
    semaphore; the walrus build in this container can't encode more than one
    wait on one instruction. Emit each wait as its own wait_ge instead."""
    nc = self.nc
    probe = nc.sync.nop(nofuse=True)
    wait_clock.add_sem_waits(probe.ins, tile.ScopedClock({None: tick_clock.global_clock}))
    si = probe.ins.sync_info
    waits = list(si.on_wait) if si is not None else []
    if len(waits) > 1:
        si.on_wait.clear()
        sem_by_num = {h.num: h for h in self.sems.allocated().values()}
        for w in waits:
            nc.sync.wait_ge(sem_by_num[w.id], w.wait_value)
    nc.sync.drain()
    nc.all_engine_barrier()
    popped = nc._tile_sem_poison_stack.pop()
    assert popped is self._sem_poison
    nc.clear_and_free_semaphores(list(self.sems.allocated().values()))
    nc.all_engine_barrier()


tile.TileContext._drain_and_barrier = _patched_drain_and_barrier

# this walrus build encodes at most this many sync waits on one instruction
MAX_WAITS = 1


def _split_excess_waits(nc, max_waits=MAX_WAITS):
    """Hoist sync waits beyond the per-instruction ISA budget onto NoOps
    inserted just before the instruction (same engine queue, so ordering
    semantics are identical). Must run AFTER Bacc.compile (its nop-fusion
    passes would re-merge the waits)."""
    for f in nc.m.functions:
        for b in f.blocks:
            ins_list = b.instructions
            out_list = []
            changed = False
            for ins in ins_list:
                si = ins.sync_info
                waits = list(si.on_wait) if si is not None else []
                if len(waits) > max_waits:
                    excess, keep = waits[:-max_waits], waits[-max_waits:]
                    for j in range(0, len(excess), max_waits):
                        nop = mybir.InstNoOp(
                            name=nc.get_next_instruction_name(), ins=[], outs=[])
                        nop.engine = ins.engine
                        nop.sync_info = mybir.SyncInfo(
                            on_wait=excess[j:j + max_waits], on_update=[])
                        out_list.append(nop)
                    ins.sync_info = mybir.SyncInfo(
                        on_wait=keep, on_update=list(si.on_update))
                    changed = True
                out_list.append(ins)
            if changed:
                b.instructions = out_list


# Problem constants (hardcoded: harness contract)
N_NODES = 100000
D = 128
HID = 128
CORES = 8

# Kernel tuning
WIN = 256         # dst window width = segment-matmul N
TILE = 128        # edge slots per tile (= matmul K)
BANK = 512        # PSUM bank width in f32 cols
CHUNK_WINS = 6    # windows per PSUM chunk (6*256 = 1536 cols = 3 banks)
GX = 6            # gather tiles per dma_gather instruction
MERGE = False     # merge gather groups across (window) regions within (chunk,k)
SCH = 4           # src chunks (int16 gather indices => table <= 32767 rows)
QN = 4            # SWDGE queues (desc-gen parallelism across Q7 cores)
PB = 8            # pre-scale pass: x-slices (128 rows each) per block
DDS = 16384       # SWDGE descriptor ring scratch bytes/partition

# crash-bisection toggles (numerics may be wrong with these set)
DBG_NOPREPASS = bool(int(os.environ.get("K_NOPREPASS", "0")))
DBG_NOXPOSE = bool(int(os.environ.get("K_NOXPOSE", "0")))
DBG_NOPB = bool(int(os.environ.get("K_NOPB", "0")))
DBG_NORAF = bool(int(os.environ.get("K_NORAF", "0")))
DBG_NOSEM = bool(int(os.environ.get("K_NOSEM", "0")))


def _prepass_blocks(r_start, r_end):
    """(row0, x_slices, partitions) blocks covering table rows
    [r_start, r_end) for the device-side pre-scale pass. Full blocks are
    PB*128 rows; the tail is a short block plus a partial-partition block."""
    blocks = []
    r = r_start
    while r + PB * 128 <= r_end:
        blocks.append((r, PB, 128))
        r += PB * 128
    rem = r_end - r
    full = rem // 128
    if full:
        blocks.append((r, full, 128))
        r += full * 128
    tail = r_end - r
    if tail:
        blocks.append((r, 1, tail))
    return blocks


def _preprocess(src, dst, n_nodes, npc, cores):
    """Host-side graph partitioning (integer index metadata only)."""
    assert n_nodes % SCH == 0
    cn = n_nodes // SCH
    assert cn < 32768, "src-chunk must fit int16 gather indices"
    src = np.asarray(src).astype(np.int64)
    dst = np.asarray(dst).astype(np.int64)
    deg = np.bincount(dst, minlength=n_nodes)
    degc = np.maximum(deg, 1)

    core_of = dst // npc
    ldst = dst - core_of * npc
    win = ldst // WIN
    kch = src // cn
    n_wins = (npc + WIN - 1) // WIN
    n_codes = n_wins * SCH
    code = win * SCH + kch

    # edges per (core, window, chunk); uniform tiles-per-(w,k) schedule
    cnt = np.zeros((cores, n_codes), np.int64)
    for c in range(cores):
        m = core_of == c
        cnt[c] = np.bincount(code[m], minlength=n_codes)
    tiles_wk = (-(-cnt.max(axis=0) // TILE)).reshape(n_wins, SCH)
    empty = tiles_wk.sum(axis=1) == 0
    tiles_wk[empty, 0] = 1  # every window writes its PSUM cols at least once

    # program tile order: psum-chunk major, then src-chunk, then window
    order = []  # (w, k) per tile
    for p0 in range(0, n_wins, CHUNK_WINS):
        p1 = min(n_wins, p0 + CHUNK_WINS)
        for k in range(SCH):
            for w in range(p0, p1):
                order.extend([(w, k)] * int(tiles_wk[w, k]))
    n_tiles = len(order)
    wk = np.array(order, np.int64)
    win_of_tile = wk[:, 0]
    # first slot of each (w,k) region (regions are contiguous in tile order)
    slot_base = np.full(n_codes, -1, np.int64)
    region_code = np.zeros(n_tiles, np.int64)
    region_start = np.zeros(n_tiles, np.int64)
    for t, (w, k) in enumerate(order):
        c_ = w * SCH + k
        if slot_base[c_] < 0:
            slot_base[c_] = t * TILE
        region_code[t] = c_
        region_start[t] = slot_base[c_] // TILE

    n_slots = n_tiles * TILE

    # gather instruction groups: balanced runs of consecutive tiles sharing
    # a src-chunk. MERGE=False: runs stay within one (w,k) region.
    # MERGE=True: runs span windows within one (psum-chunk, k) — pad slots
    # (gidx=0, offs=-1) contribute nothing, so interior pads are legal.
    def _split_run(k, ta, tb):
        nt = tb - ta
        ng = -(-nt // GX)
        base, rem = divmod(nt, ng)
        a = ta
        for i in range(ng):
            sz = base + (1 if i < rem else 0)
            groups.append((k, a, a + sz))
            a += sz
        assert a == tb

    groups = []  # (k, t_start, t_end)
    t = 0
    for p0 in range(0, n_wins, CHUNK_WINS):
        p1 = min(n_wins, p0 + CHUNK_WINS)
        for k in range(SCH):
            run0 = t
            for w in range(p0, p1):
                nt_r = int(tiles_wk[w, k])
                if not MERGE and nt_r:
                    _split_run(k, t, t + nt_r)
                t += nt_r
            if MERGE and t > run0:
                _split_run(k, run0, t)
    assert t == n_tiles

    padn = n_wins * WIN

    per_core = []
    for c in range(cores):
        m = np.nonzero(core_of == c)[0]
        nm = len(m)
        src_c, win_c, kch_c, code_c, ldst_c = src[m], win[m], kch[m], code[m], ldst[m]
        # group by (w,k), ascending src within the group (gather locality)
        o = np.lexsort((src_c, kch_c, win_c))
        codes_s = code_c[o]
        gstart = np.searchsorted(codes_s, np.arange(n_codes))
        rank = np.arange(nm) - gstart[codes_s]
        slot = slot_base[codes_s] + rank

        gidx = np.zeros(n_slots, np.int16)    # pads: row 0 of the chunk
        offs = np.full(n_slots, -1.0, np.float32)  # pads never match iota

        gidx[slot] = (src_c[o] - kch_c[o] * cn).astype(np.int16)
        offs[slot] = (ldst_c[o] - win_c[o] * WIN).astype(np.float32)

        # per-group real descriptor counts: the gather ucode only processes
        # num_idxs_reg descriptors, so tail pads (gidx 0 / offs -1) are
        # skipped. Groups lie within one (w,k) region (MERGE=False), whose
        # real edges fill slots from the region start.
        real_of_code = cnt[c]
        counts = np.zeros(len(groups), np.int32)
        for gi_, (k_, ta_, tb_) in enumerate(groups):
            code_ = region_code[ta_]
            r0_ = region_start[ta_]
            real = int(real_of_code[code_])
            c_real = int(np.clip(real - (ta_ - r0_) * TILE,
                                 16, (tb_ - ta_) * TILE))
            counts[gi_] = -(-c_real // 16) * 16  # idx wrap-row granularity

        # [n_slots] -> [128, n_tiles]: slot j of tile t at [j, t]
        offs_t = np.ascontiguousarray(
            offs.reshape(n_tiles, TILE).T).astype(ml_dtypes.bfloat16)

        # int16 idx wrap for dma_gather: within-instruction idx i at
        # [i % 16, i // 16], replicated across the 8 16-partition groups.
        # Instruction = run of whole tiles, so per-tile 8-col blocks suffice.
        a = gidx.reshape(n_tiles, 8, 16)          # [t, i//16, i%16]
        wrapped = np.transpose(a, (2, 0, 1)).reshape(16, n_tiles * 8)
        gidx16 = np.ascontiguousarray(np.tile(wrapped, (8, 1)))  # [128, 8*ET]

        # dst-degree clip, permuted so [p, x] = degc[own_row x*128+p]
        nx_d = padn // 128
        degd = np.ones(padn, np.float32)
        degd[:npc] = degc[c * npc:(c + 1) * npc].astype(np.float32)
        degdp = np.ascontiguousarray(degd.reshape(nx_d, 128).T)  # [128, nx_d]

        per_core.append(dict(gidx16=gidx16, offs=offs_t, degdp=degdp,
                             counts=counts.reshape(1, -1)))

    # src-degree clip for the table pre-scale pass, laid out in pre-pass
    # block emission order: column j = j-th x-slice emitted, value
    # degc[r0 + x*pcnt + p] (pad 1.0 on partial-partition tails)
    flat = degc.astype(np.float32)
    cols = []
    for k in range(SCH):
        for (r0, xs, pcnt) in _prepass_blocks(k * cn, (k + 1) * cn):
            for x in range(xs):
                col = np.ones(128, np.float32)
                col[:pcnt] = flat[r0 + x * pcnt: r0 + (x + 1) * pcnt]
                cols.append(col)
    degsp = np.ascontiguousarray(np.stack(cols, axis=1))

    return dict(
        groups=groups,
        n_wins=n_wins,
        n_tiles=n_tiles,
        win_of_tile=win_of_tile,
        per_core=per_core,
        degsp=degsp,
    )


def _build_program(sched, n_nodes, npc, split_waits=True):
    """Build the single SPMD Bass/Tile program (identical for all cores)."""
    n_wins = sched["n_wins"]
    n_tiles = sched["n_tiles"]
    win_of_tile = sched["win_of_tile"]
    cn = n_nodes // SCH
    padn = n_wins * WIN            # padded local dst count (cols of out^T)
    n_chunks = -(-n_wins // CHUNK_WINS)
    nxb = sched["degsp"].shape[1]
    nx_d = padn // 128

    nc = bacc.Bacc("TRN2", target_bir_lowering=False, num_swdge_queues=QN,
                   dynamic_dma_scratch_size=DDS)
    hb = nc.declare_dram_parameter("hb", [n_nodes, D], BF16, isOutput=False)
    hself = nc.declare_dram_parameter("hself", [padn, D], BF16, isOutput=False)
    gidx_p = nc.declare_dram_parameter("gidx16", [TILE, 8 * n_tiles], I16, isOutput=False)
    offs_p = nc.declare_dram_parameter("offs", [TILE, n_tiles], BF16, isOutput=False)
    wt_p = nc.declare_dram_parameter("wt", [2 * D, HID], BF16, isOutput=False)
    bias_p = nc.declare_dram_parameter("bias_c", [HID, 1], F32, isOutput=False)
    degsp_p = nc.declare_dram_parameter("degsp", [128, nxb], F32, isOutput=False)
    degdp_p = nc.declare_dram_parameter("degdp", [128, nx_d], F32, isOutput=False)
    out_p = nc.declare_dram_parameter("out", [HID, padn], F32, isOutput=True)

    # pre-scaled node table (hs = h * rsqrt(deg_src)), device-written
    hs = nc.dram_tensor("hs", [n_nodes, D], BF16, kind="Internal")
    # dst-norm row staging (layout shuffle [128, nx_d] -> flat [padn])
    rd_dram = nc.dram_tensor("rd_dram", [nx_d, 128], BF16, kind="Internal")

    with tile.TileContext(nc) as tc:
        with (
            tc.tile_pool(name="const", bufs=1) as const,
            tc.tile_pool(name="pp", bufs=3) as pp,
            tc.tile_pool(name="g", bufs=8) as gpool,
            tc.tile_pool(name="oh", bufs=8) as ohpool,
            tc.tile_pool(name="nd", bufs=2) as ndpool,
            tc.tile_pool(name="slab", bufs=2) as slab,
            tc.tile_pool(name="y", bufs=6) as ypool,
            tc.tile_pool(name="aggps", bufs=2, space="PSUM") as agg_ps,
            tc.tile_pool(name="scrps", bufs=2, space="PSUM") as scr_ps,
        ):
            # ---- constants / metadata ----
            gidx_sb = const.tile([TILE, 8 * n_tiles], I16)
            nc.sync.dma_start(gidx_sb[:], gidx_p[:])
            offs_sb = const.tile([TILE, n_tiles], BF16)
            nc.sync.dma_start(offs_sb[:], offs_p[:])

            w1_sb = const.tile([D, HID], BF16)
            nc.sync.dma_start(w1_sb[:], wt_p[0:D, :])
            w2_sb = const.tile([D, HID], BF16)
            nc.sync.dma_start(w2_sb[:], wt_p[D:2 * D, :])
            bias_sb = const.tile([HID, 1], F32)
            nc.sync.dma_start(bias_sb[:], bias_p[:])
            ones_sb = const.tile([128, 128], F32)
            nc.vector.memset(ones_sb[:], 1.0)

            # iota replicated per gather-tile: [128, GX, WIN], value = col
            iota_i = const.tile([128, GX, WIN], I32)
            nc.gpsimd.iota(iota_i[:], pattern=[[0, GX], [1, WIN]], base=0,
                           channel_multiplier=0)
            iota_b = const.tile([128, GX, WIN], BF16)
            nc.vector.tensor_copy(iota_b[:], iota_i[:])

            # ---- src-norm: rs[p, x] = rsqrt(degc[x*128+p]) ----
            degs_sb = const.tile([128, nxb], F32)
            nc.sync.dma_start(degs_sb[:], degsp_p[:])
            rs_sb = const.tile([128, nxb], F32)
            if DBG_NORAF:
                nc.vector.reciprocal(rs_sb[:], degs_sb[:])
            else:
                nc.vector.reciprocal_approx_fast(rs_sb[:], degs_sb[:])
            nc.scalar.sqrt(rs_sb[:], rs_sb[:])

            # ---- dst-norm row vector normd1[0, j] = rsqrt(degc_dst[j]) ----
            degd_sb = const.tile([128, nx_d], F32)
            nc.sync.dma_start(degd_sb[:], degdp_p[:])
            rdf_sb = const.tile([128, nx_d], F32)
            if DBG_NORAF:
                nc.vector.reciprocal(rdf_sb[:], degd_sb[:])
            else:
                nc.vector.reciprocal_approx_fast(rdf_sb[:], degd_sb[:])
            nc.scalar.sqrt(rdf_sb[:], rdf_sb[:])
            rd_sb = const.tile([128, nx_d], BF16)
            nc.vector.tensor_copy(rd_sb[:], rdf_sb[:])
            normd1 = const.tile([1, padn], BF16)
            if not DBG_NOPB:
                # normd1[0, x*128+p] = rd_sb[p, x], via a DRAM staging bounce
                # (the write->read hazard is covered by the drain below).
                # Pre-pass DMAs ride the Activation HWDGE queue so they don't
                # serialize behind the big metadata loads on the SP queue.
                nc.scalar.dma_start(
                    rd_dram[:, :].rearrange("x p -> p x"), rd_sb[:])

            # ---- pre-scale pass: hs[r] = hb[r] * rs[r], per src-chunk ----
            # DRAM write->read hazards (hs, rd_dram) are invisible to Tile:
            # after each chunk's stores, drain the SP HWDGE queue (blocks SP
            # until all its issued DMAs complete), then a token DMA whose
            # SBUF tile gates the Pool engine via a tracked dependency right
            # before the chunk's first gather.
            gather_src = hb if DBG_NOPREPASS else hs
            toks = {}
            x0 = 0  # running degsp column (block emission order)
            for k in range(SCH):
                if DBG_NOPREPASS:
                    break
                for (r0, xs, pcnt) in _prepass_blocks(k * cn, (k + 1) * cn):
                    nrows = xs * pcnt
                    tin = pp.tile([128, PB, D], BF16, tag="ppin")
                    nc.scalar.dma_start(
                        tin[:pcnt, :xs, :],
                        hb[r0:r0 + nrows, :].rearrange("(x p) f -> p x f", p=pcnt),
                    )
                    tout = pp.tile([128, PB, D], BF16, tag="ppout")
                    nc.vector.tensor_tensor(
                        out=tout[:pcnt, :xs, :], in0=tin[:pcnt, :xs, :],
                        in1=rs_sb[:pcnt, x0:x0 + xs].unsqueeze(2).broadcast_to(
                            [pcnt, xs, D]),
                        op=mybir.AluOpType.mult)
                    nc.scalar.dma_start(
                        hs[r0:r0 + nrows, :].rearrange("(x p) f -> p x f", p=pcnt),
                        tout[:pcnt, :xs, :],
                    )
                    x0 += xs
                nc.scalar.drain()
                if k == 0 and not DBG_NOPB:
                    # rd_dram bounce was issued before chunk 0's stores
                    nc.scalar.dma_start(
                        normd1[:].rearrange("o (x p) -> o x p", p=128),
                        rd_dram[:, :])
                tok = const.tile([1, 8], BF16, tag=f"tok{k}")
                nc.scalar.dma_start(tok[:], hb[0:1, 0:8])
                toks[k] = tok

            def gate_chunk(k):
                if k in toks:
                    tok2 = const.tile([1, 8], BF16, tag=f"tokc{k}")
                    nc.gpsimd.tensor_copy(tok2[:], toks.pop(k)[:])

            # shared num_idxs registers for dma_gather (one per distinct size)
            ni_regs = {}

            def ni_reg(n):
                if n not in ni_regs:
                    r = nc.gpsimd.alloc_register(f"nireg{len(ni_regs)}")
                    nc.gpsimd.reg_mov(r, n)
                    ni_regs[n] = r
                return ni_regs[n]

            group_by_start = {g[1]: (gi, g) for gi, g in enumerate(sched["groups"])}

            tile_of_chunk = [[] for _ in range(n_chunks)]
            for t in range(n_tiles):
                tile_of_chunk[int(win_of_tile[t]) // CHUNK_WINS].append(t)

            # ---- main loop over dst chunks ----
            for ch in range(n_chunks):
                w0 = ch * CHUNK_WINS
                w1 = min(n_wins, w0 + CHUNK_WINS)
                cw = (w1 - w0) * WIN
                col0 = w0 * WIN
                tlist = tile_of_chunk[ch]
                assert tlist == list(range(tlist[0], tlist[-1] + 1))
                t0c, t1c = tlist[0], tlist[-1] + 1

                # first/last program-order touch per psum bank in this chunk
                bank_of = [(int(win_of_tile[t]) - w0) * WIN // BANK for t in tlist]
                first_of_bank, last_of_bank = {}, {}
                for t, bk in zip(tlist, bank_of):
                    first_of_bank.setdefault(bk, t)
                    last_of_bank[bk] = t

                pagg = agg_ps.tile([128, CHUNK_WINS * WIN], F32, tag="pagg")

                # gather groups: region-aligned runs, up to GX tiles
                g0 = t0c
                while g0 < t1c:
                    gi, (k, ta, gend) = group_by_start[g0]
                    assert ta == g0
                    gt = gend - g0
                    gate_chunk(k)
                    G = gpool.tile([128, GX, D], BF16, tag="G")
                    nc.gpsimd.dma_gather(
                        out_ap=G[:, :gt, :],
                        in_ap=gather_src[k * cn:(k + 1) * cn, :],
                        idxs_ap=gidx_sb[:, 8 * g0:8 * gend],
                        num_idxs=TILE * gt,
                        num_idxs_reg=ni_reg(TILE * gt),
                        elem_size=D,
                        queue_num=gi % QN,
                    )
                    oh = ohpool.tile([128, GX, WIN], BF16, tag="oh")
                    off_bc = offs_sb[:, g0:gend].unsqueeze(2).broadcast_to([128, gt, WIN])
                    nc.vector.tensor_tensor(out=oh[:, :gt, :], in0=off_bc,
                                            in1=iota_b[:, :gt, :],
                                            op=mybir.AluOpType.is_equal)
                    for x in range(gt):
                        t = g0 + x
                        col = (int(win_of_tile[t]) - w0) * WIN
                        bk = bank_of[t - t0c]
                        nc.tensor.matmul(
                            pagg[:, col:col + WIN],
                            lhsT=G[:, x, :],
                            rhs=oh[:, x, :],
                            start=(first_of_bank[bk] == t),
                            stop=(last_of_bank[bk] == t),
                            skip_group_check=True,
                        )
                    g0 = gend

                # dst-norm column scale for this chunk, broadcast to 128 parts
                ndbc = ndpool.tile([128, CHUNK_WINS * WIN], BF16, tag="ndbc")
                if DBG_NOPB:
                    nc.vector.memset(ndbc[:, :cw], 1.0)
                else:
                    nc.gpsimd.partition_broadcast(
                        ndbc[:, :cw], normd1[:, col0:col0 + cw])

                # evacuate agg chunk: cast to bf16 with dst-norm applied
                aggT = slab.tile([128, CHUNK_WINS * WIN], BF16, tag="aggT")
                nc.vector.tensor_tensor(out=aggT[:, :cw], in0=pagg[:, :cw],
                                        in1=ndbc[:, :cw], op=mybir.AluOpType.mult)

                # h^T slab for this chunk's dst rows via HW DMA transpose
                hT = slab.tile([128, CHUNK_WINS * WIN], BF16, tag="hT")
                if DBG_NOXPOSE:
                    nc.vector.memset(hT[:, :cw], 0.0)
                else:
                    nc.sync.dma_start(hT[:, :cw], hself[col0:col0 + cw, :],
                                      transpose=True)

                # out^T = W1.T @ h^T + W2.T @ agg^T ; + bias; L2 normalize; store
                for bs in range(0, cw, BANK):
                    bw = min(BANK, cw - bs)
                    po = scr_ps.tile([128, BANK], F32, tag="scr")
                    nc.tensor.matmul(po[:, :bw], lhsT=w1_sb[:], rhs=hT[:, bs:bs + bw],
                                     start=True, stop=False)
                    nc.tensor.matmul(po[:, :bw], lhsT=w2_sb[:], rhs=aggT[:, bs:bs + bw],
                                     start=False, stop=True)
                    y = ypool.tile([128, BANK], F32, tag="y")
                    nc.scalar.activation(y[:, :bw], po[:, :bw],
                                         mybir.ActivationFunctionType.Identity,
                                         bias=bias_sb[:])
                    z = ypool.tile([128, BANK], F32, tag="z")
                    nc.scalar.square(z[:, :bw], y[:, :bw])
                    pr = scr_ps.tile([128, BANK], F32, tag="scr")
                    nc.tensor.matmul(pr[:, :bw], lhsT=ones_sb[:], rhs=z[:, :bw],
                                     start=True, stop=True)
                    rs = ypool.tile([128, BANK], F32, tag="rs")
                    nc.vector.reciprocal_approx_fast(rs[:, :bw], pr[:, :bw])
                    nc.scalar.sqrt(rs[:, :bw], rs[:, :bw])
                    of = ypool.tile([128, BANK], F32, tag="of")
                    nc.vector.tensor_tensor(out=of[:, :bw], in0=y[:, :bw],
                                            in1=rs[:, :bw], op=mybir.AluOpType.mult)
                    nc.sync.dma_start(out_p[:, col0 + bs:col0 + bs + bw], of[:, :bw])

    nc.finalize()
    if split_waits:
        _split_excess_waits(nc)
    return nc


def _run(h, weight, bias, src, dst, n_nodes, npc, cores, trace=False):
    sched = _preprocess(src, dst, n_nodes, npc, cores)
    nc = _build_program(sched, n_nodes, npc)

    padn = sched["n_wins"] * WIN
    h = np.asarray(h, dtype=np.float32)
    hb = h.astype(ml_dtypes.bfloat16)
    wt = np.asarray(weight, dtype=np.float32).astype(ml_dtypes.bfloat16)
    bias_c = np.ascontiguousarray(np.asarray(bias, dtype=np.float32).reshape(HID, 1))

    in_maps = []
    for c in range(cores):
        pc = sched["per_core"][c]
        hself = np.zeros((padn, D), dtype=ml_dtypes.bfloat16)
        hself[:npc] = hb[c * npc:(c + 1) * npc]
        in_maps.append(dict(
            hb=hb, hself=hself,
            gidx16=pc["gidx16"], offs=pc["offs"],
            wt=wt, bias_c=bias_c,
            degsp=sched["degsp"], degdp=pc["degdp"],
        ))

    res = run_bass_kernel_spmd(nc, in_maps, core_ids=list(range(cores)), trace=trace)
    out = np.empty((cores * npc, HID), dtype=np.float32)
    for c in range(cores):
        out[c * npc:(c + 1) * npc] = res.results[c]["out"][:, :npc].T
    return out, res


def kernel(h, weight, bias, src, dst):
    out, _ = _run(h, weight, bias, src, dst, N_NODES, N_NODES // CORES, CORES)
    return out
